# revision 2
# baseline (speedup 1.0000x reference)
"""Correlation layer (FlowNet-style, max_displacement=4) on 8 Trainium2 cores.

Sharding: data-parallel over batch — 16 samples, 2 per NeuronCore.

Per-core Bass kernel, per (sample b, row h):
  The 81-displacement correlation row is computed on the PE as
  out[w, (w', dy)] = sum_c f1[c, h, w] * f2pad[c, h+dy, w'],
  i.e. a [128w x 648] matmul rectangle per w-half (w' band of 72, dy
  interleaved innermost), accumulated over two 128-channel chunks into
  PSUM. The useful values sit on the staircase (w' = w+dx, dx in 0..8):
  col 9*(w - w_half_base) + 9*dx + dy of the half-rect.
  DVE evacuates half 0, ScalarE half 1 (scale 1/256, cast bf16), and four
  window DMAs [32, 360] ship the staircase bands to DRAM. The host peels
  the diagonals with a zero-copy as_strided view and assembles
  [B, 81, H, W] float32.
"""
import sys

sys.path.insert(0, "/opt/trn_rl_repo")

import numpy as np

PAD = 4
D = 9
B, C, H, W = 16, 256, 64, 128
PADH, PADW = H + 2 * PAD, W + 2 * PAD  # 72, 136
N_CORES = 8
NB = B // N_CORES  # samples per core
WINC = 360  # window cols: 9*31 + 81

_prog_cache = {}


def _build_program():
    from concourse import bacc, mybir, tile

    nc = bacc.Bacc("TRN2", target_bir_lowering=False, debug=False)
    f1 = nc.declare_dram_parameter("f1", [NB, C, H, W], mybir.dt.float32, isOutput=False)
    f2p = nc.declare_dram_parameter(
        "f2p", [NB, C, PADH, PADW], mybir.dt.float32, isOutput=False
    )
    outw = nc.declare_dram_parameter(
        "outw", [NB, H, 4, 32, WINC], mybir.dt.bfloat16, isOutput=True
    )

    with tile.TileContext(nc) as tc:
        with (
            tc.tile_pool(name="fin", bufs=2) as fin_pool,
            tc.tile_pool(name="psA", bufs=2, space="PSUM") as psA_pool,
            tc.tile_pool(name="psB", bufs=2, space="PSUM") as psB_pool,
            tc.tile_pool(name="evA", bufs=4) as evA_pool,
            tc.tile_pool(name="evB", bufs=4) as evB_pool,
        ):
            for b in range(NB):
                f1s = []
                f2s = []
                for ck in range(2):
                    t1 = fin_pool.tile(
                        [128, H, W], mybir.dt.bfloat16, tag="f1s", name=f"f1s_{b}_{ck}"
                    )
                    nc.gpsimd.dma_start(out=t1[:], in_=f1[b, 128 * ck : 128 * (ck + 1)])
                    f1s.append(t1)
                    t2 = fin_pool.tile(
                        [128, PADH, PADW],
                        mybir.dt.bfloat16,
                        tag="f2s",
                        name=f"f2s_{b}_{ck}",
                    )
                    nc.gpsimd.dma_start(out=t2[:], in_=f2p[b, 128 * ck : 128 * (ck + 1)])
                    f2s.append(t2)
                for h in range(H):
                    psA = psA_pool.tile(
                        [128, 2, 512], mybir.dt.float32, tag="psA", name=f"psA_{b}_{h}"
                    )
                    psB = psB_pool.tile(
                        [128, 2, 512], mybir.dt.float32, tag="psB", name=f"psB_{b}_{h}"
                    )
                    ps = [psA, psB]
                    for ck in range(2):
                        lhsT = f1s[ck][:, h, :]  # [128c, 128w]
                        for half in range(2):
                            for g in range(2):
                                w0 = 64 * half + 36 * g
                                # moving [128c, 36w', 9dy], dy innermost
                                rhs = f2s[ck][:, h : h + 9, w0 : w0 + 36].transpose(
                                    [0, 2, 1]
                                )
                                nc.tensor.matmul(
                                    ps[half][:, g, 0:324],
                                    lhsT=lhsT,
                                    rhs=rhs,
                                    start=(ck == 0),
                                    stop=(ck == 1),
                                )
                    evA = evA_pool.tile(
                        [128, 2, 324], mybir.dt.bfloat16, tag="evA", name=f"evA_{b}_{h}"
                    )
                    evB = evB_pool.tile(
                        [128, 2, 324], mybir.dt.bfloat16, tag="evB", name=f"evB_{b}_{h}"
                    )
                    nc.vector.tensor_scalar_mul(evA[:], ps[0][:, :, 0:324], 1.0 / C)
                    nc.scalar.activation(
                        evB[:],
                        ps[1][:, :, 0:324],
                        mybir.ActivationFunctionType.Copy,
                        scale=1.0 / C,
                    )
                    evAf = evA[:].rearrange("p a b -> p (a b)")  # [128, 648]
                    evBf = evB[:].rearrange("p a b -> p (a b)")
                    nc.sync.dma_start(out=outw[b, h, 0], in_=evAf[0:32, 0:360])
                    nc.sync.dma_start(out=outw[b, h, 1], in_=evAf[32:64, 288:648])
                    nc.sync.dma_start(out=outw[b, h, 2], in_=evBf[64:96, 0:360])
                    nc.sync.dma_start(out=outw[b, h, 3], in_=evBf[96:128, 288:648])
    nc.compile()
    return nc


def get_program():
    if "nc" not in _prog_cache:
        _prog_cache["nc"] = _build_program()
    return _prog_cache["nc"]


def extract_output(outw_np):
    """outw [NB, H, 4, 32, 360] bf16 -> [NB, 81, H, W] f32 (diagonal peel)."""
    wf = np.ascontiguousarray(outw_np).astype(np.float32)
    s = wf.strides
    diag = np.lib.stride_tricks.as_strided(
        wf,
        shape=(NB, H, 4, 32, 81),
        strides=(s[0], s[1], s[2], s[3] + 9 * s[4], s[4]),
    )
    diag = diag.reshape(NB, H, W, D, D)  # [b, h, w, dx, dy]
    return np.ascontiguousarray(
        diag.transpose(0, 4, 3, 1, 2).reshape(NB, D * D, H, W)
    )


def run_spmd(f1_np, f2p_np, **kwargs):
    from concourse.bass_utils import run_bass_kernel_spmd

    nc = get_program()
    in_maps = [
        {"f1": f1_np[NB * i : NB * (i + 1)], "f2p": f2p_np[NB * i : NB * (i + 1)]}
        for i in range(N_CORES)
    ]
    return run_bass_kernel_spmd(nc, in_maps, list(range(N_CORES)), **kwargs)


def kernel(features1, features2):
    f1 = np.ascontiguousarray(np.asarray(features1, dtype=np.float32))
    f2 = np.ascontiguousarray(np.asarray(features2, dtype=np.float32))
    f2p = np.pad(f2, ((0, 0), (0, 0), (PAD, PAD), (PAD, PAD)))
    res = run_spmd(f1, f2p)
    shards = [extract_output(res.results[i]["outw"]) for i in range(N_CORES)]
    return np.concatenate(shards, axis=0)


if __name__ == "__main__":
    rng = np.random.default_rng(0)
    a = rng.standard_normal((B, C, H, W), dtype=np.float32)
    b = rng.standard_normal((B, C, H, W), dtype=np.float32)
    y = kernel(features1=a, features2=b)
    print("out:", y.shape, y.dtype)
